# revision 10
# baseline (speedup 1.0000x reference)
"""Gemma3 sliding-window attention on 8 TRN2 NeuronCores via Bass/Tile.

Sharding: core c = b*4 + g  (b = batch, g = head-group):
  - q-heads {2g, 2g+1}, kv-head g, batch b  -> zero redundant projection work
  - column-shard wq/wk/wv, RoPE+RMSNorm local, blocked sliding-window
    attention, then a 4-rank bf16 AllGather of the attention outputs
    (concat over heads) per batch group, and a column-shard of wo.
All tensor-engine work in bf16 (f32 PSUM accumulation); softmax in f32->bf16.

Layout is fully transposed on-chip: Q^T/K^T are [head_dim, tokens] straight
out of the projection matmuls, V is [tokens, head_dim], scores are S^T
[keys, queries], attention output is O^T [head_dim, tokens] which directly
feeds the o-proj contraction. No transposes anywhere.
"""
import sys
import numpy as np

if "/opt/trn_rl_repo" not in sys.path:
    sys.path.insert(0, "/opt/trn_rl_repo")

from contextlib import ExitStack

import ml_dtypes
import concourse.bass as bass
import concourse.mybir as mybir
import concourse.tile as tile
from concourse import bacc
from concourse import bass_utils

BF16 = mybir.dt.bfloat16
F32 = mybir.dt.float32
NPBF16 = ml_dtypes.bfloat16

B, S, HID = 2, 2048, 2560
NH, NKV, HD = 8, 4, 256
SCALE = 256.0 ** -0.5
EPS = 1e-6
WIN = 1024
NCORES = 8
KT = HID // 128          # 20 k-tiles over hidden dim
QB = 512                 # query block (free dim of score matmuls)
NQB = S // QB            # 4 query blocks
TM = S // 128            # 16 token tiles of 128
NHID_LOC = 640           # per-core slice of o-proj output columns

REPLICA_GROUPS = [[0, 1, 2, 3], [4, 5, 6, 7]]


def _emit(nc):
    AF = mybir.ActivationFunctionType

    xT = nc.dram_tensor("xT", [HID, S], BF16, kind="ExternalInput")
    wq = nc.dram_tensor("wq", [HID, 512], BF16, kind="ExternalInput")
    wk = nc.dram_tensor("wk", [HID, 256], BF16, kind="ExternalInput")
    wv = nc.dram_tensor("wv", [HID, 256], BF16, kind="ExternalInput")
    wo = nc.dram_tensor("wo", [NH * HD, NHID_LOC], BF16, kind="ExternalInput")
    cosT = nc.dram_tensor("cosT", [HD, S], BF16, kind="ExternalInput")
    rsinT = nc.dram_tensor("rsinT", [HD, S], BF16, kind="ExternalInput")
    qnw = nc.dram_tensor("qnw", [HD, 1], F32, kind="ExternalInput")
    knw = nc.dram_tensor("knw", [HD, 1], F32, kind="ExternalInput")
    maskb = nc.dram_tensor("maskb", [128, 1920], BF16, kind="ExternalInput")
    out = nc.dram_tensor("out", [S, NHID_LOC], F32, kind="ExternalOutput")

    xT_r = xT.rearrange("(t p) w -> p t w", p=128)        # [128, 20, 2048]
    wq_r = wq.rearrange("(t p) n -> p t n", p=128)        # [128, 20, 512]
    wk_r = wk.rearrange("(t p) n -> p t n", p=128)
    wv_r = wv.rearrange("(t p) n -> p t n", p=128)
    wo_r = wo.rearrange("(t p) n -> p t n", p=128)        # [128, 16, 640]
    cosT_r = cosT.rearrange("(d p) w -> p d w", p=128)    # [128, 2, 2048]
    rsinT_r = rsinT.rearrange("(d p) w -> p d w", p=128)
    qnw_r = qnw.rearrange("(d p) o -> p (d o)", p=128)    # [128, 2]
    knw_r = knw.rearrange("(d p) o -> p (d o)", p=128)

    with ExitStack() as ctx:
        tc = ctx.enter_context(tile.TileContext(nc))
        ec = ctx.enter_context
        constp = ec(tc.tile_pool(name="const", bufs=1))
        persist = ec(tc.tile_pool(name="persist", bufs=1))
        dram = ec(tc.tile_pool(name="dram", bufs=1, space="DRAM"))

        # ---- constants (live for the whole kernel) ----
        wq_s = constp.tile([128, KT, 512], BF16)
        wk_s = constp.tile([128, KT, 256], BF16)
        wv_s = constp.tile([128, KT, 256], BF16)
        mask_s = constp.tile([128, 1920], BF16)
        ones_s = constp.tile([128, 1], BF16)
        nc.sync.dma_start(wq_s[:], wq_r[:])
        nc.sync.dma_start(wk_s[:], wk_r[:])
        nc.sync.dma_start(wv_s[:], wv_r[:])
        nc.sync.dma_start(mask_s[:], maskb[:])
        nc.vector.memset(ones_s[:], 1.0)

        # persistent activations
        qT_s = persist.tile([128, 2, 2, S], BF16)   # [p, head, hd-chunk, tok]
        kT_s = persist.tile([128, 2, S], BF16)      # [p, hd-chunk, tok]
        v_s = persist.tile([128, TM, 256], BF16)    # [p(tok), tok-tile, hd]

        # ================= phase 1: projections + RMSNorm + RoPE ==========
        p1 = ExitStack()
        scr = p1.enter_context(tc.tile_pool(name="scr", bufs=2))
        tiny = p1.enter_context(tc.tile_pool(name="tiny", bufs=2))
        xtp = p1.enter_context(tc.tile_pool(name="xt", bufs=1))
        ph1c = p1.enter_context(tc.tile_pool(name="ph1c", bufs=1))
        psq = p1.enter_context(tc.tile_pool(name="psq", bufs=2, space="PSUM"))
        psk = p1.enter_context(tc.tile_pool(name="psk", bufs=2, space="PSUM"))
        psv = p1.enter_context(tc.tile_pool(name="psv", bufs=2, space="PSUM"))
        psss = p1.enter_context(tc.tile_pool(name="psss", bufs=1, space="PSUM"))

        cos_s = ph1c.tile([128, 2, S], BF16)
        rsin_s = ph1c.tile([128, 2, S], BF16)
        qn_s = ph1c.tile([128, 2], F32)
        kn_s = ph1c.tile([128, 2], F32)
        nc.sync.dma_start(cos_s[:], cosT_r[:])
        nc.sync.dma_start(rsin_s[:], rsinT_r[:])
        nc.sync.dma_start(qn_s[:], qnw_r[:])
        nc.sync.dma_start(kn_s[:], knw_r[:])

        def rms_rope(ps_pair, nw_s, dst, dst_h, t0):
            """ps_pair: two [128, QB] f32 PSUM chunks of one head's ^T proj.
            Normalize (RMS over the 256 partition dims), scale by per-dim
            weight, apply RoPE, write bf16 into dst[:, (dst_h,) d, t0:t0+QB].
            """
            sqs = []
            for d in range(2):
                sq = scr.tile([128, QB], BF16, tag="sq")
                nc.scalar.activation(sq[:], ps_pair[d][:], AF.Square)
                sqs.append(sq)
            ss = psss.tile([1, QB], F32, tag="ss")
            nc.tensor.matmul(ss[:], ones_s[:], sqs[0][:], start=True, stop=False)
            nc.tensor.matmul(ss[:], ones_s[:], sqs[1][:], start=False, stop=True)
            ssn = tiny.tile([1, QB], F32, tag="ssn")
            nc.scalar.activation(ssn[:], ss[:], AF.Copy, bias=EPS, scale=1.0 / HD)
            rin = tiny.tile([1, QB], F32, tag="rin")
            nc.vector.reciprocal(rin[:], ssn[:])
            rr = tiny.tile([1, QB], BF16, tag="rr")
            nc.scalar.activation(rr[:], rin[:], AF.Sqrt)
            rrb = scr.tile([128, QB], BF16, tag="rrb")
            nc.gpsimd.partition_broadcast(rrb[:], rr[:])
            qrm = []
            for d in range(2):
                tq = scr.tile([128, QB], BF16, tag="tq")
                nc.vector.tensor_scalar(
                    tq[:], ps_pair[d][:], nw_s[:, d:d + 1], None,
                    mybir.AluOpType.mult,
                )
                qr = scr.tile([128, QB], BF16, tag="qrm")
                nc.vector.tensor_mul(qr[:], tq[:], rrb[:])
                qrm.append(qr)
            for d in range(2):
                a = scr.tile([128, QB], BF16, tag="ra")
                b = scr.tile([128, QB], BF16, tag="rb")
                nc.vector.tensor_mul(a[:], qrm[d][:], cos_s[:, d, t0:t0 + QB])
                nc.vector.tensor_mul(b[:], qrm[1 - d][:], rsin_s[:, d, t0:t0 + QB])
                if dst_h is None:
                    dslice = dst[:, d, t0:t0 + QB]
                else:
                    dslice = dst[:, dst_h, d, t0:t0 + QB]
                nc.vector.tensor_add(dslice, a[:], b[:])

        for half in range(2):
            xt_s = xtp.tile([128, KT, S // 2], BF16, tag="xt")
            nc.sync.dma_start(
                xt_s[:], xT_r[:, :, half * (S // 2):(half + 1) * (S // 2)]
            )
            for tc_i in range(2):
                t0 = half * (S // 2) + tc_i * QB
                lo = tc_i * QB
                xs = xt_s[:, :, lo:lo + QB]
                # K^T for this token chunk
                kps = []
                for d in range(2):
                    pk = psk.tile([128, QB], F32, tag="pk")
                    for kt in range(KT):
                        nc.tensor.matmul(
                            pk[:], wk_s[:, kt, d * 128:(d + 1) * 128],
                            xs[:, kt, :], start=(kt == 0), stop=(kt == KT - 1),
                        )
                    kps.append(pk)
                rms_rope(kps, kn_s, kT_s, None, t0)
                # Q^T per head
                for h in range(2):
                    qps = []
                    for d in range(2):
                        pq = psq.tile([128, QB], F32, tag="pq")
                        c = 2 * h + d
                        for kt in range(KT):
                            nc.tensor.matmul(
                                pq[:], wq_s[:, kt, c * 128:(c + 1) * 128],
                                xs[:, kt, :], start=(kt == 0), stop=(kt == KT - 1),
                            )
                        qps.append(pq)
                    rms_rope(qps, qn_s, qT_s, h, t0)
                # V (natural layout) for the 4 token tiles in this chunk
                for mm in range(4):
                    m = (t0 // 128) + mm
                    pv = psv.tile([128, 256], F32, tag="pv")
                    for kt in range(KT):
                        nc.tensor.matmul(
                            pv[:], xs[:, kt, mm * 128:(mm + 1) * 128],
                            wv_s[:, kt, :], start=(kt == 0), stop=(kt == KT - 1),
                        )
                    nc.vector.tensor_copy(v_s[:, m, :], pv[:])

        p1.close()

        # ========== phases 2+3 per query block: attn -> AllGather -> o-proj
        p2 = ExitStack()
        wop = p2.enter_context(tc.tile_pool(name="wop", bufs=1))
        esb = p2.enter_context(tc.tile_pool(name="esb", bufs=3))
        scr2 = p2.enter_context(tc.tile_pool(name="scr2", bufs=2))
        attp = p2.enter_context(tc.tile_pool(name="attp", bufs=2))
        agp = p2.enter_context(tc.tile_pool(name="agp", bufs=1))
        outpp = p2.enter_context(tc.tile_pool(name="outp", bufs=2))
        pss = p2.enter_context(tc.tile_pool(name="pss", bufs=2, space="PSUM"))
        pso = p2.enter_context(tc.tile_pool(name="pso", bufs=1, space="PSUM"))
        psse = p2.enter_context(tc.tile_pool(name="psse", bufs=1, space="PSUM"))
        psop = p2.enter_context(tc.tile_pool(name="psop", bufs=1, space="PSUM"))

        wo_s = wop.tile([128, TM, NHID_LOC], BF16)
        nc.sync.dma_start(wo_s[:], wo_r[:])

        for qb in range(NQB):
            q0 = qb * QB
            kt_lo = max(0, (q0 - WIN) // 128)
            kt_hi = (q0 + QB - 1) // 128
            att = attp.tile([128, 4, QB], BF16, tag="att")
            for h in range(2):
                o_ps = pso.tile([128, 2, QB], F32, tag="o")
                se_ps = psse.tile([1, QB], F32, tag="se")
                for kt in range(kt_lo, kt_hi + 1):
                    k0 = kt * 128
                    d_off = q0 - k0
                    s_ps = pss.tile([128, QB], F32, tag="s")
                    for d in range(2):
                        nc.tensor.matmul(
                            s_ps[:], kT_s[:, d, k0:k0 + 128],
                            qT_s[:, h, d, q0:q0 + QB],
                            start=(d == 0), stop=(d == 1),
                        )
                    e_s = esb.tile([128, QB], BF16, tag="e")
                    nc.scalar.activation(e_s[:], s_ps[:], AF.Exp)
                    if not (128 <= d_off <= 512):
                        nc.vector.tensor_mul(
                            e_s[:], e_s[:],
                            mask_s[:, 384 + d_off:384 + d_off + QB],
                        )
                    first, last = kt == kt_lo, kt == kt_hi
                    for d in range(2):
                        nc.tensor.matmul(
                            o_ps[:, d, :], v_s[:, kt, d * 128:(d + 1) * 128],
                            e_s[:], start=first, stop=last,
                        )
                    nc.tensor.matmul(
                        se_ps[:], ones_s[:], e_s[:], start=first, stop=last,
                    )
                rc = scr2.tile([1, QB], F32, tag="rc")
                nc.vector.reciprocal(rc[:], se_ps[:])
                rcb = scr2.tile([128, QB], F32, tag="rcb")
                nc.gpsimd.partition_broadcast(rcb[:], rc[:])
                for d in range(2):
                    nc.vector.tensor_mul(att[:, 2 * h + d, :], o_ps[:, d, :], rcb[:])
            # AllGather this block's attention outputs across the batch group
            agin = dram.tile([512, QB], BF16, tag=f"agin{qb}")
            agout = dram.tile([NH * HD, QB], BF16, tag=f"agout{qb}")
            agin_r = agin.rearrange("(c p) w -> c p w", p=128)
            for c in range(4):
                nc.sync.dma_start(agin_r[c], att[:, c, :])
            nc.gpsimd.collective_compute(
                "AllGather",
                mybir.AluOpType.bypass,
                replica_groups=REPLICA_GROUPS,
                ins=[agin[:]],
                outs=[agout[:]],
            )
            ag_s = agp.tile([128, TM, QB], BF16, tag="ag")
            nc.sync.dma_start(ag_s[:], agout.rearrange("(t p) w -> p t w", p=128))
            # o-proj for this block's 4 token tiles
            for mm in range(4):
                m = qb * 4 + mm
                po = psop.tile([128, NHID_LOC], F32, tag="po")
                for kt2 in range(TM):
                    lhs = ag_s[:, kt2, mm * 128:(mm + 1) * 128]
                    nc.tensor.matmul(
                        po[:, 0:512], lhs, wo_s[:, kt2, 0:512],
                        start=(kt2 == 0), stop=(kt2 == TM - 1),
                    )
                    nc.tensor.matmul(
                        po[:, 512:NHID_LOC], lhs, wo_s[:, kt2, 512:NHID_LOC],
                        start=(kt2 == 0), stop=(kt2 == TM - 1),
                    )
                ot = outpp.tile([128, NHID_LOC], F32, tag="ot")
                nc.vector.tensor_copy(ot[:, 0:512], po[:, 0:512])
                nc.vector.tensor_copy(ot[:, 512:NHID_LOC], po[:, 512:NHID_LOC])
                nc.sync.dma_start(out[m * 128:(m + 1) * 128, :], ot[:])
        p2.close()

    nc.compile()
    return nc


_NC = None


def _build():
    global _NC
    if _NC is None:
        _NC = _emit(
            bacc.Bacc("TRN2", target_bir_lowering=False, debug=False,
                      num_devices=NCORES)
        )
    return _NC


def _host_prep(hidden_states, cos, sin, wq, wk, wv, wo, q_norm_w, k_norm_w):
    """Build the 8 per-core input maps (numpy, bf16 where device expects bf16)."""
    f32 = np.float32
    qn = ((1.0 + q_norm_w.astype(f32)) * SCALE).reshape(HD, 1)
    kn = (1.0 + k_norm_w.astype(f32)).reshape(HD, 1)
    # rsin: [-sin_firsthalf, +sin_secondhalf] so rope = q*cos + q[swap]*rsin
    # mask band: maskb[kk, y] = 1 iff 0 <= (y-384) - kk < WIN
    kk = np.arange(128)[:, None]
    y = np.arange(1920)[None, :]
    maskb = ((y - 384 - kk >= 0) & (y - 384 - kk < WIN)).astype(NPBF16)

    in_maps = []
    for c in range(NCORES):
        b, g = divmod(c, 4)
        sin_b = sin[b].astype(f32)
        rsin = np.concatenate([-sin_b[:, :128], sin_b[:, 128:]], axis=1)
        in_maps.append({
            "xT": np.ascontiguousarray(hidden_states[b].T).astype(NPBF16),
            "wq": np.ascontiguousarray(
                wq[:, 2 * g * HD:(2 * g + 2) * HD]).astype(NPBF16),
            "wk": np.ascontiguousarray(wk[:, g * HD:(g + 1) * HD]).astype(NPBF16),
            "wv": np.ascontiguousarray(wv[:, g * HD:(g + 1) * HD]).astype(NPBF16),
            "wo": np.ascontiguousarray(
                wo[:, g * NHID_LOC:(g + 1) * NHID_LOC]).astype(NPBF16),
            "cosT": np.ascontiguousarray(cos[b].T).astype(NPBF16),
            "rsinT": np.ascontiguousarray(rsin.T).astype(NPBF16),
            "qnw": qn,
            "knw": kn,
            "maskb": maskb,
        })
    return in_maps


class _Runner:
    """Compile the Bass module to a reusable 8-device PJRT executable
    (mirrors bass2jax.run_bass_via_pjrt but keeps the jitted fn for
    repeated steady-state invocation)."""

    def __init__(self, nc):
        import jax
        from jax.sharding import Mesh, PartitionSpec
        try:
            from jax import shard_map as _sm
            shard_map = _sm.shard_map if hasattr(_sm, "shard_map") else _sm
        except Exception:
            from jax.experimental.shard_map import shard_map
        from concourse import bass2jax
        from concourse.bass2jax import _bass_exec_p

        bass2jax.install_neuronx_cc_hook()
        self.jax = jax
        self.nc = nc
        part_name = (nc.partition_id_tensor.name
                     if nc.partition_id_tensor else None)
        in_names, out_names, out_avals = [], [], []
        for alloc in nc.m.functions[0].allocations:
            if not isinstance(alloc, mybir.MemoryLocationSet):
                continue
            name = alloc.memorylocations[0].name
            if alloc.kind == "ExternalInput":
                if name != part_name:
                    in_names.append(name)
            elif alloc.kind == "ExternalOutput":
                out_names.append(name)
                out_avals.append(jax.core.ShapedArray(
                    tuple(alloc.tensor_shape), mybir.dt.np(alloc.dtype)))
        self.in_names, self.out_names, self.out_avals = in_names, out_names, out_avals
        all_names = list(in_names) + list(out_names)
        if part_name is not None:
            all_names.append(part_name)

        def _body(*args):
            operands = list(args)
            if part_name is not None:
                operands.append(bass2jax.partition_id_tensor())
            outs = _bass_exec_p.bind(
                *operands,
                out_avals=tuple(out_avals),
                in_names=tuple(all_names),
                out_names=tuple(out_names),
                lowering_input_output_aliases=(),
                sim_require_finite=True,
                sim_require_nnan=True,
                nc=nc,
            )
            return tuple(outs)

        devices = jax.devices()[:NCORES]
        self.mesh = Mesh(np.asarray(devices), ("core",))
        n_args = len(in_names) + len(out_names)
        self.fn = jax.jit(
            shard_map(
                _body, mesh=self.mesh,
                in_specs=(PartitionSpec("core"),) * n_args,
                out_specs=(PartitionSpec("core"),) * len(out_names),
                check_vma=False,
            ),
            keep_unused=True,
        )
        self.sharding = jax.sharding.NamedSharding(
            self.mesh, PartitionSpec("core"))
        self.zeros = [
            jax.device_put(
                np.zeros((NCORES * a.shape[0], *a.shape[1:]), a.dtype),
                self.sharding)
            for a in out_avals
        ]

    def put(self, in_maps):
        concat = [
            np.concatenate([np.asarray(in_maps[c][n]) for c in range(NCORES)],
                           axis=0)
            for n in self.in_names
        ]
        return [self.jax.device_put(a, self.sharding) for a in concat]

    def run(self, in_dev):
        outs = self.fn(*in_dev, *self.zeros)
        return [o.block_until_ready() for o in outs]

    def results(self, outs):
        per_core = []
        for c in range(NCORES):
            m = {}
            for i, n in enumerate(self.out_names):
                a = self.out_avals[i]
                m[n] = np.asarray(outs[i]).reshape(NCORES, *a.shape)[c]
            per_core.append(m)
        return per_core


_RUNNER = None


def _get_runner():
    global _RUNNER
    if _RUNNER is None:
        _RUNNER = _Runner(_build())
    return _RUNNER


def kernel(hidden_states, cos, sin, wq, wk, wv, wo, q_norm_w, k_norm_w):
    r = _get_runner()
    in_maps = _host_prep(hidden_states, cos, sin, wq, wk, wv, wo,
                         q_norm_w, k_norm_w)
    res = r.results(r.run(r.put(in_maps)))
    out = np.empty((B, S, HID), np.float32)
    for b in range(B):
        out[b] = np.concatenate(
            [res[b * 4 + g]["out"] for g in range(4)], axis=1
        )
    return out
